# revision 22
# baseline (speedup 1.0000x reference)
"""CTC loss forward on Trainium2 (Bass/Tile), batch-sharded over 8 cores.

Algorithm: probability-domain CTC alpha recurrence restructured as a loop
over the 257 extended states; for each state the full time series within a
t-chunk satisfies a first-order linear recurrence computed by ONE
tensor_tensor_scan along the free (time) axis, with sequences on partitions.
fp32 range is managed by a self-computed gauge: per-chunk re-centering of
every state row from the live carry, plus block-shared slopes estimated
from the previous chunk's realized decay.

Distribution: data-parallel over the batch dim N — each of the 8 cores runs
the full T-step recurrence for its 8 sequences (partitions 0..7). One SPMD
program serves all cores; the length-dependent extraction is data-driven via
per-core 0/1 mask tensors and an on-device chunk counter. Emissions upload
only the 129 distinct rows (blank + targets) of the 257 extended states —
targets in fp8-e4m3, blanks in bf16 — packed host-side in per-(chunk, block)
slab order so every DMA is contiguous per partition.

The dominant dispatch cost in this environment is per *stream* instruction
(~75us each), so the whole recurrence is expressed with nested hardware
loops (tc.For_i) over (chunk, block, row-pair) with dynamic AP offsets; only
the 4 warmup chunks + the first 64-chunk are emitted statically. The scalar
engine's activation mis-handles dynamic bias APs, so the per-block bias
exp(nbm[b]) is applied as a separate vector multiply.

  T, N, C, S = 1024, 64, 128, 128 ; Sx = 2*S+1 = 257
  output: scalar f32 loss = -logsumexp_n alpha[il_n-1, n, 2*tl_n-1]
"""
import math
import os
import sys
from contextlib import ExitStack

import numpy as np

sys.path.insert(0, "/opt/trn_rl_repo")

import concourse.bass as bass
import concourse.tile as tile
from concourse import bacc, mybir
from concourse.bass import ds
from concourse.bass_utils import run_bass_kernel_spmd

F32 = mybir.dt.float32
BF16 = mybir.dt.bfloat16
EM_DT = mybir.dt.float8e4             # target emissions (rel-err budget ~3e-3)
EM_NP = "float8_e4m3"                 # ml_dtypes name for host packing
BL_DT = mybir.dt.bfloat16             # blank series (shared by all even states)
BL_NP = "bfloat16"
AF = mybir.ActivationFunctionType
OP = mybir.AluOpType

T, N, C, S = 1024, 64, 128, 128
Sx = 2 * S + 1
NCORES = 8
NP_CORE = N // NCORES                 # sequences (partitions) per core

SCHED = [16, 16, 32] + [64] * 15      # t-chunk lengths, sum == T
NWARM = 3                             # warmup chunks emitted statically
BLK = 32                              # slope-sharing block size along s
JBLK = BLK // 2                       # target rows per s-block (odd states)
LOGBIAS = 18.0                        # recenter q to exp(-LOGBIAS) at chunk starts
CG_FLOOR = -19.0                      # log floor for the cc scale cgamma
SL0 = -5.33                           # warmup slope guess (chunk 0)
CH0B = 18.0                           # chunk-0 gauge offset
NEGBIG = -1.0e30


def _chunk_starts(sched):
    t0s, t = [], 0
    for L in sched:
        t0s.append(t)
        t += L
    return t0s


def _slab_offsets(sched):
    toff, boff = {}, {}
    pos = bpos = 0
    for ci, L in enumerate(sched):
        Ls = L - (1 if ci == 0 else 0)
        boff[ci] = bpos
        bpos += Ls
        for b in range(8):
            toff[(ci, b)] = pos
            pos += JBLK * Ls
    return toff, boff, pos, bpos


def _extract_plan(il, tl, t0s, t_total=T):
    """Per-sequence extraction site: (chunk, srow, local col)."""
    per_n = {}
    for n in range(len(il)):
        te = min(int(il[n]), t_total) - 1
        srow = 2 * int(tl[n]) - 1
        ci = max(i for i, t0 in enumerate(t0s) if t0 <= te)
        per_n[n] = (ci, srow, te - t0s[ci] + 1)
        # extraction is handled inside the dynamic chunk loop
        assert ci >= NWARM + 1
    return per_n


def build_program(sched=SCHED, t_total=T):
    """Build the SPMD Bass program. Fully input-independent: extraction is
    driven by the uploaded masks, so no length specialization at all."""
    t0s = _chunk_starts(sched)
    assert t0s[-1] + sched[-1] == t_total
    Lmax = max(sched)
    L1max = Lmax + 1
    toff, boff, tgt_tot, bl_tot = _slab_offsets(sched)
    nloop = len(sched) - NWARM - 1     # chunks run by the dynamic loop
    ci0 = NWARM + 1                    # first dynamic chunk
    QW = Sx * L1max                    # flat Q width (64-chunk layout)

    NP_ = NP_CORE
    nc = bacc.Bacc("TRN2", target_bir_lowering=False, debug=False)

    etgt_d = nc.dram_tensor("etgt", [NP_, tgt_tot], EM_DT, kind="ExternalInput").ap()
    ebl_d = nc.dram_tensor("ebl", [NP_, bl_tot], BL_DT, kind="ExternalInput").ap()
    mlog_d = nc.dram_tensor("mlog", [NP_, Sx], F32, kind="ExternalInput").ap()
    qinit_d = nc.dram_tensor("qinit", [NP_, Sx], F32, kind="ExternalInput").ap()
    iota_d = nc.dram_tensor("iotat", [NP_, Lmax], F32, kind="ExternalInput").ap()
    rm65_d = nc.dram_tensor("rm65", [NP_, QW], BF16, kind="ExternalInput").ap()
    rm257_d = nc.dram_tensor("rm257", [NP_, Sx], F32, kind="ExternalInput").ap()
    cgate_d = nc.dram_tensor("cgate", [NP_, nloop], F32, kind="ExternalInput").ap()
    tfac_d = nc.dram_tensor("tfac", [NP_, 1], F32, kind="ExternalInput").ap()
    v_d = nc.dram_tensor("v_out", [NP_, 1], F32, kind="ExternalOutput").ap()

    with tile.TileContext(nc) as tc, ExitStack() as ctx:
        state = ctx.enter_context(tc.tile_pool(name="state", bufs=1))

        Q = state.tile([NP_, QW], F32)
        rm65 = state.tile([NP_, QW], BF16)
        evb = state.tile([NP_, QW], BF16)
        OffAcc = state.tile([NP_, Sx], F32)
        slope = state.tile([NP_, Sx], F32)
        mlog_t = state.tile([NP_, Sx], F32)
        qinit_t = state.tile([NP_, Sx], F32)
        iota_t = state.tile([NP_, Lmax], F32)
        rm257 = state.tile([NP_, Sx], F32)
        cgate_t = state.tile([NP_, nloop], F32)
        tfac_t = state.tile([NP_, 1], F32)
        zero_t = state.tile([NP_, Lmax], F32)
        ones_t = state.tile([NP_, BLK], F32)
        # gauge aux
        lq = state.tile([NP_, Sx], F32)
        lqb = state.tile([NP_, Sx], F32)
        slr = state.tile([NP_, Sx], F32)
        offtmp = state.tile([NP_, Sx], F32)
        d1g = state.tile([NP_, Sx], F32)
        d2t = state.tile([NP_, Sx], F32)
        d2m = state.tile([NP_, Sx], F32)
        dom = state.tile([NP_, Sx], F32)
        logcg = state.tile([NP_, Sx], F32)
        aexp = state.tile([NP_, Sx], F32)
        bexp = state.tile([NP_, Sx], F32)
        a_t = state.tile([NP_, Sx], F32)
        b_t = state.tile([NP_, Sx], F32)
        cg = state.tile([NP_, Sx], F32)
        invcg = state.tile([NP_, Sx], F32)
        qi0 = state.tile([NP_, Sx], F32)
        bm = state.tile([NP_, 9], F32)
        nbm = state.tile([NP_, 9], F32)
        ebias = state.tile([NP_, 9], F32)
        qcl = state.tile([NP_, Sx], F32)
        bclip = state.tile([NP_, 1], F32)
        # row-loop working tiles (fixed; For_i back-edge serializes iterations)
        eblb = state.tile([NP_, Lmax], BL_DT)
        pbexp = state.tile([NP_, Lmax], F32)
        ebuf = state.tile([NP_, JBLK * Lmax], EM_DT)
        eraw = state.tile([NP_, JBLK * Lmax], F32)
        Eodd = state.tile([NP_, JBLK * L1max], F32)
        ebkS = state.tile([NP_, L1max], F32)
        dslt = state.tile([NP_, 1], F32)
        gt = state.tile([NP_, Lmax], F32)
        gsert = state.tile([NP_, Lmax], F32)
        cct = state.tile([NP_, Lmax], F32)
        t1t = state.tile([NP_, Lmax], F32)
        t2t = state.tile([NP_, Lmax], F32)
        rt = state.tile([NP_, Lmax], F32)
        # extraction accumulators
        evs = state.tile([NP_, Sx], F32)
        red1 = state.tile([NP_, 1], F32)
        red2 = state.tile([NP_, 1], F32)
        vqrun = state.tile([NP_, 1], F32)
        voffrun = state.tile([NP_, 1], F32)
        vslrun = state.tile([NP_, 1], F32)
        vln = state.tile([NP_, 1], F32)
        vtmp = state.tile([NP_, 1], F32)
        vout_t = state.tile([NP_, 1], F32)
        nblk = (Sx + BLK - 1) // BLK  # 9

        # one-time setup
        nc.sync.dma_start(mlog_t[:], mlog_d)
        nc.sync.dma_start(qinit_t[:], qinit_d)
        nc.sync.dma_start(iota_t[:], iota_d)
        nc.sync.dma_start(rm65[:], rm65_d)
        nc.sync.dma_start(rm257[:], rm257_d)
        nc.sync.dma_start(cgate_t[:], cgate_d)
        nc.sync.dma_start(tfac_t[:], tfac_d)
        nc.vector.memset(zero_t[:], 0.0)
        nc.vector.memset(ones_t[:], 1.0)
        nc.vector.memset(OffAcc[:], CH0B)
        nc.vector.memset(slope[:], SL0)
        nc.vector.memset(ebkS[:, 0:1], 1.0)
        nc.vector.memset(vqrun[:], 0.0)
        nc.vector.memset(voffrun[:], 0.0)
        nc.vector.memset(vslrun[:], 0.0)

        def emit_gauge(ci_static_first, Lp, Lp1):
            """Per-chunk gauge update. All APs static."""
            if not ci_static_first:
                Qpv = Q[:, : Sx * Lp1].rearrange("p (s l) -> p s l", l=Lp1)
                nc.vector.tensor_scalar(
                    qcl[:], Qpv[:, :, Lp1 - 1], 2.0 ** -8, 1e-36, OP.mult, OP.max)
                nc.scalar.activation(lq[:], qcl[:], AF.Ln)
                nc.vector.tensor_scalar_add(lqb[:], lq[:], LOGBIAS + 8.0 * math.log(2.0))
                nc.vector.scalar_tensor_tensor(
                    slr[:], lqb[:], 1.0 / Lp, slope[:], OP.mult, OP.add)
                nc.vector.scalar_tensor_tensor(
                    offtmp[:], slope[:], float(Lp), OffAcc[:], OP.mult, OP.add)
                nc.vector.tensor_add(OffAcc[:], offtmp[:], lqb[:])
                nc.vector.tensor_reduce(
                    bm[:, 0:8], slr[:, 0:256].rearrange("p (b j) -> p b j", j=BLK),
                    mybir.AxisListType.X, OP.add)
                nc.vector.tensor_scalar_mul(bm[:, 0:8], bm[:, 0:8], 1.0 / BLK)
                nc.vector.tensor_copy(bm[:, 8:9], slr[:, 256:257])
                for b in range(1, nblk):
                    nc.vector.scalar_tensor_tensor(
                        bclip[:], bm[:, b - 1:b], -1.2, bm[:, b:b + 1], OP.add, OP.max)
                    nc.vector.scalar_tensor_tensor(
                        bm[:, b:b + 1], bm[:, b - 1:b], 1.2, bclip[:], OP.add, OP.min)
                for b in range(nblk):
                    src = max(b - 1, 0)
                    lo, hi = b * BLK, min((b + 1) * BLK, Sx)
                    nc.scalar.mul(slope[:, lo:hi], ones_t[:, : hi - lo], bm[:, src:src + 1])
                    nc.scalar.mul(nbm[:, b:b + 1], bm[:, src:src + 1], -1.0)
            else:
                for b in range(nblk):
                    nc.scalar.mul(nbm[:, b:b + 1], ones_t[:, 0:1], -SL0)

            nc.vector.memset(d1g[:, 0:1], NEGBIG)
            nc.vector.tensor_sub(d1g[:, 1:Sx], OffAcc[:, 0:Sx - 1], OffAcc[:, 1:Sx])
            nc.vector.memset(d2m[:, 0:2], NEGBIG)
            nc.vector.tensor_sub(d2t[:, 2:Sx], OffAcc[:, 0:Sx - 2], OffAcc[:, 2:Sx])
            nc.vector.tensor_add(d2m[:, 2:Sx], d2t[:, 2:Sx], mlog_t[:, 2:Sx])
            nc.vector.tensor_max(dom[:], d1g[:], d2m[:])
            nc.vector.tensor_scalar(
                logcg[:], dom[:], CG_FLOOR, 80.0, OP.max, OP.min)
            nc.vector.tensor_sub(aexp[:], d1g[:], logcg[:])
            nc.scalar.activation(a_t[:], aexp[:], AF.Exp)
            nc.vector.memset(a_t[:, 0:1], 0.0)
            nc.vector.tensor_sub(bexp[:], d2m[:], logcg[:])
            nc.scalar.activation(b_t[:], bexp[:], AF.Exp)
            nc.vector.memset(b_t[:, 0:2], 0.0)
            nc.scalar.activation(cg[:], logcg[:], AF.Exp)
            nc.scalar.activation(invcg[:], logcg[:], AF.Exp, scale=-1.0)
            nc.scalar.activation(ebias[:], nbm[:], AF.Exp)

        def emit_chunk_rows(ci_static, Ls, cbase, ebloff):
            """Row loop of one chunk. ci_static is an int for the statically
            emitted chunks and None inside the dynamic chunk loop (then cbase/
            ebloff are ScalarValue expressions and the chunk is 64 long)."""
            L1 = Ls + 1
            first = ci_static == 0
            Qv = Q[:, : Sx * L1].rearrange("p (s l) -> p s l", l=L1)
            Eov = Eodd[:, : JBLK * L1].rearrange("p (j l) -> p j l", l=L1)
            erawv = eraw[:, : JBLK * Ls].rearrange("p (j l) -> p j l", l=Ls)
            bstride = JBLK * Ls

            def Qrow(s, c0, n):
                return Q[:, ds(s * L1 + c0, n)]

            def col(t_, s):
                return t_[:, ds(s, 1)]

            # qi0 = invcg * carry (scan initial; data0[0] == 1)
            if first:
                nc.vector.tensor_mul(qi0[:], invcg[:], qinit_t[:])
                nc.sync.dma_start(Qv[:, :, 0], qinit_d)
            else:
                nc.vector.tensor_scalar_mul(qi0[:], invcg[:], math.exp(-LOGBIAS))
                nc.vector.memset(Qv[:, :, 0], math.exp(-LOGBIAS))

            nc.sync.dma_start(eblb[:, 0:Ls], ebl_d[:, ds(ebloff, Ls)])
            nc.scalar.activation(pbexp[:, 0:Ls], eblb[:, 0:Ls], AF.Exp)
            nc.vector.memset(Eov[:, :, 0], 1.0)

            def load_block(bi):
                nc.sync.dma_start(
                    ebuf[:, 0: JBLK * Ls], etgt_d[:, ds(cbase + bi * bstride, bstride)])
                nc.scalar.activation(eraw[:, 0: JBLK * Ls], ebuf[:, 0: JBLK * Ls], AF.Exp)
                nc.vector.tensor_scalar_mul(Eov[:, :, 1:L1], erawv[:, :, :], col(ebias, bi))
                nc.vector.tensor_scalar_mul(ebkS[:, 1:L1], pbexp[:, 0:Ls], col(ebias, bi))

            def make_gser(bi):
                nc.vector.tensor_sub(
                    dslt[:], slope[:, ds(bi * BLK - 1, 1)], slope[:, ds(bi * BLK, 1)])
                nc.vector.tensor_scalar_mul(gt[:, 0:Ls], iota_t[:, 0:Ls], dslt[:])
                nc.scalar.activation(gsert[:, 0:Ls], gt[:, 0:Ls], AF.Exp)

            def even_row(s, gser=False, cc_zero=False):
                if cc_zero:
                    ccv = zero_t[:, 0:Ls]
                else:
                    nc.vector.tensor_scalar_mul(cct[:, 0:Ls], Qrow(s - 1, 0, Ls), col(a_t, s))
                    if gser:
                        nc.vector.tensor_mul(t2t[:, 0:Ls], cct[:, 0:Ls], gsert[:, 0:Ls])
                    ccv = (t2t if gser else cct)[:, 0:Ls]
                nc.vector.tensor_tensor_scan(
                    rt[:, 0:Ls], ebkS[:, 0:Ls], ccv, col(qi0, s), OP.mult, OP.add)
                nc.vector.scalar_tensor_tensor(
                    Qrow(s, 1, Ls), rt[:, 0:Ls], col(cg, s), ebkS[:, 1:L1],
                    OP.mult, OP.mult)

            def odd_row(s, p, gser=False, has2=True):
                if has2:
                    nc.vector.tensor_scalar_mul(t1t[:, 0:Ls], Qrow(s - 2, 0, Ls), col(b_t, s))
                    if gser:
                        nc.vector.tensor_mul(t2t[:, 0:Ls], t1t[:, 0:Ls], gsert[:, 0:Ls])
                    nc.vector.scalar_tensor_tensor(
                        cct[:, 0:Ls], Qrow(s - 1, 0, Ls), col(a_t, s),
                        (t2t if gser else t1t)[:, 0:Ls], OP.mult, OP.add)
                else:
                    nc.vector.tensor_scalar_mul(cct[:, 0:Ls], Qrow(s - 1, 0, Ls), col(a_t, s))
                nc.vector.tensor_tensor_scan(
                    rt[:, 0:Ls], Eodd[:, ds(p * L1, Ls)], cct[:, 0:Ls], col(qi0, s),
                    OP.mult, OP.add)
                nc.vector.scalar_tensor_tensor(
                    Qrow(s, 1, Ls), rt[:, 0:Ls], col(cg, s), Eodd[:, ds(p * L1 + 1, Ls)],
                    OP.mult, OP.mult)

            # block 0 (rows 0,1 special)
            load_block(0)
            even_row(0, cc_zero=True)
            odd_row(1, 0, has2=False)
            with tc.For_i(1, 16, 1) as p:
                even_row(2 * p)
                odd_row(2 * p + 1, p)
            # blocks 1..7
            if first:
                with tc.For_i(1, 8, 1) as bi:
                    load_block(bi)
                    with tc.For_i(0, 16, 1) as p:
                        even_row(bi * 32 + 2 * p)
                        odd_row(bi * 32 + 2 * p + 1, p)
            else:
                with tc.For_i(1, 8, 1) as bi:
                    load_block(bi)
                    make_gser(bi)
                    even_row(bi * 32, gser=True)
                    odd_row(bi * 32 + 1, 0, gser=True)
                    with tc.For_i(1, 16, 1) as p:
                        even_row(bi * 32 + 2 * p)
                        odd_row(bi * 32 + 2 * p + 1, p)
            # block 8: s=256
            nc.vector.tensor_scalar_mul(ebkS[:, 1:L1], pbexp[:, 0:Ls], ebias[:, 8:9])
            if first:
                even_row(256)
            else:
                make_gser(8)
                even_row(256, gser=True)

        # ---- warmup chunks + first 64-chunk: static ----
        for ci in range(NWARM + 1):
            L = sched[ci]
            tb = 1 if ci == 0 else 0
            emit_gauge(ci == 0, sched[ci - 1], (sched[ci - 1] - (1 if ci == 1 else 0)) + 1)
            emit_chunk_rows(ci, L - tb, toff[(ci, 0)], boff[ci])

        # ---- dynamic loop over the remaining identical 64-chunks ----
        cb0 = toff[(ci0, 0)]
        bl0 = boff[ci0]

        def chunk_body(cj):
            emit_gauge(False, 64, 65)
            emit_chunk_rows(None, 64, cb0 + cj * (8 * JBLK * 64), bl0 + cj * 64)
            # extraction: each partition grabs its value in its gated chunk
            gcol = cgate_t[:, ds(cj, 1)]
            nc.vector.tensor_mul(evb[:], Q[:], rm65[:])
            nc.vector.tensor_reduce(red1[:], evb[:], mybir.AxisListType.X, OP.add)
            nc.vector.tensor_mul(red2[:], red1[:], gcol)
            nc.vector.tensor_add(vqrun[:], vqrun[:], red2[:])
            nc.vector.tensor_mul(evs[:], OffAcc[:], rm257[:])
            nc.vector.tensor_reduce(red1[:], evs[:], mybir.AxisListType.X, OP.add)
            nc.vector.tensor_mul(red2[:], red1[:], gcol)
            nc.vector.tensor_add(voffrun[:], voffrun[:], red2[:])
            nc.vector.tensor_mul(evs[:], slope[:], rm257[:])
            nc.vector.tensor_reduce(red1[:], evs[:], mybir.AxisListType.X, OP.add)
            nc.vector.tensor_mul(red2[:], red1[:], gcol)
            nc.vector.tensor_add(vslrun[:], vslrun[:], red2[:])

        if os.environ.get("CTC_UNROLL_CHUNKS", "0") == "1":
            for cj in range(nloop):
                chunk_body(cj)
        else:
            with tc.For_i(0, nloop, 1) as cj:
                chunk_body(cj)

        # ---- final: v = ln(vq) + voff + vsl*tfac ----
        nc.scalar.activation(vln[:], vqrun[:], AF.Ln)
        nc.vector.scalar_tensor_tensor(
            vtmp[:], vslrun[:], tfac_t[:, 0:1], voffrun[:], OP.mult, OP.add)
        nc.vector.tensor_add(vout_t[:], vtmp[:], vln[:])
        nc.sync.dma_start(v_d, vout_t[:])

    nc.compile()
    return nc


def host_prepare(log_probs, targets, input_lengths, target_lengths,
                 sched=SCHED, t_total=T):
    """Pack per-core input maps. Core c owns sequences c*8 .. c*8+7."""
    import ml_dtypes
    em_np = np.dtype(getattr(ml_dtypes, EM_NP))
    bl_np = np.dtype(getattr(ml_dtypes, BL_NP))
    bf_np = np.dtype(ml_dtypes.bfloat16)
    lp = np.asarray(log_probs, np.float32)[:t_total]
    tg = np.asarray(targets).astype(np.int32)
    il = np.minimum(np.asarray(input_lengths).astype(np.int64), t_total)
    tl = np.asarray(target_lengths).astype(np.int64)
    n = lp.shape[1]
    t0s = _chunk_starts(sched)
    toff, boff, tgt_tot, bl_tot = _slab_offsets(sched)
    per_n = _extract_plan(il, tl, t0s, t_total)
    Lmax = max(sched)
    L1max = Lmax + 1
    QW = Sx * L1max

    ext = np.zeros((n, Sx), np.int32)
    ext[:, 1::2] = tg
    skip = np.zeros((n, Sx), bool)
    skip[:, 2:] = ext[:, 2:] != ext[:, :-2]
    mlog = np.where(skip, 0.0, NEGBIG).astype(np.float32)

    # lp is [T, n, C]; gather targets along C -> [T, n, S] -> [n, S, T]
    g = np.take_along_axis(lp, np.broadcast_to(tg[None], (t_total, n, S)), axis=2)
    etgt_full = np.ascontiguousarray(g.transpose(1, 2, 0)).astype(em_np)
    ebl_full = np.ascontiguousarray(lp[:, :, 0].T).astype(bl_np)  # [n, T]

    etgt = np.empty((n, tgt_tot), em_np)
    ebl = np.empty((n, bl_tot), bl_np)
    for ci, L in enumerate(sched):
        tb = 1 if ci == 0 else 0
        Ls = L - tb
        t0 = t0s[ci]
        ebl[:, boff[ci]: boff[ci] + Ls] = ebl_full[:, t0 + tb: t0 + L]
        for b in range(8):
            off = toff[(ci, b)]
            etgt[:, off: off + JBLK * Ls] = etgt_full[
                :, b * JBLK:(b + 1) * JBLK, t0 + tb: t0 + L].reshape(n, -1)

    e0 = np.exp(lp[0][np.arange(n)[:, None], ext]).astype(np.float32)
    p0 = np.ones((n, Sx), np.float32)
    p0[:, :2] = e0[:, :2]
    qinit = (p0 * np.float32(math.exp(-(CH0B + SL0)))).astype(np.float32)
    iota = np.tile(np.arange(Lmax, dtype=np.float32), (n, 1))

    nloop = len(sched) - NWARM - 1
    ci0 = NWARM + 1
    rm65 = np.zeros((n, QW), bf_np)
    rm257 = np.zeros((n, Sx), np.float32)
    cgate = np.zeros((n, nloop), np.float32)
    tfac = np.zeros((n, 1), np.float32)
    for i in range(n):
        ci, srow, c = per_n[i]
        rm65[i, srow * L1max + c] = 1.0
        rm257[i, srow] = 1.0
        cgate[i, ci - ci0] = 1.0
        tfac[i, 0] = c

    in_maps = []
    for c in range(NCORES):
        sl = slice(c * NP_CORE, (c + 1) * NP_CORE)
        in_maps.append({
            "etgt": etgt[sl], "ebl": ebl[sl], "mlog": mlog[sl],
            "qinit": qinit[sl], "iotat": iota[sl], "rm65": rm65[sl],
            "rm257": rm257[sl], "cgate": cgate[sl], "tfac": tfac[sl],
        })
    return in_maps, il, tl


LAST_EXEC_NS = None
_NC_CACHE = None


def kernel(log_probs, targets, input_lengths, target_lengths):
    global LAST_EXEC_NS, _NC_CACHE
    in_maps, ilc, tl = host_prepare(log_probs, targets, input_lengths, target_lengths)
    if _NC_CACHE is None:
        _NC_CACHE = build_program()
    nc = _NC_CACHE
    trace = os.environ.get("CTC_TRACE", "0") == "1"
    res = run_bass_kernel_spmd(
        nc, in_maps, core_ids=list(range(NCORES)), trace=trace)
    LAST_EXEC_NS = res.exec_time_ns
    v = np.concatenate(
        [res.results[c]["v_out"].reshape(-1) for c in range(NCORES)]
    ).astype(np.float64)
    m0 = v.max()
    loss = -(m0 + np.log(np.exp(v - m0).sum()))
    return np.float32(loss)
